# revision 4
# baseline (speedup 1.0000x reference)
"""Bipartite GNN message passing on 8 Trainium2 NeuronCores.

Reference math (fp32):
    adjT = normalize(adj.T); adjN = normalize(adj)      # row-normalize each
    s1 = (adjT @ u0) Ws ; u1 = (adjN @ s0) Wu
    s2 = (adjT @ u1) Ws ; u2 = (adjN @ s1) Wu
    s3 = (adjT @ u2) Ws ; u3 = (adjN @ s2) Wu
    return s3, u3

Everything is linear, so the per-layer 128x128 weight multiplies commute past
the adjacency multiplies and the diagonal normalizations:
    s3 = D_c^-1 A^T D_r^-1 A D_c^-1 A^T u0 @ (Ws Wu Ws)
    u3 = D_r^-1 A D_c^-1 A^T D_r^-1 A s0 @ (Wu Ws Wu)
where A = adj, D_r = diag(rowsum), D_c = diag(colsum).

Distribution: row-shard A across 8 cores (adj_m = A[1024m:1024(m+1), :]).
Per core, 4 passes over the local shard (cached in SBUF as fp16 after pass 1):
    P1: t1p = adj_m^T @ [u0_m | 1]        -> ReduceScatter(+)  (t1, colsum)
    P2: t2_m = adj_m @ t1n ; r1_m = adj_m @ s0    (local; needs AllGather t1n)
    P3: t3p = adj_m^T @ t2n_m ; r2p = adj_m^T @ r1n_m -> ReduceScatter(+)
    P4: r3_m = adj_m @ r2n                        (needs AllGather r2n)
    sub_out_m = (t3 * 1/colsum_m) @ (Ws Wu Ws)   [fp32]
    uni_out_m = (r3 * 1/rowsum_m) @ (Wu Ws Wu)   [fp32]
adj^T-products consume the natural-orientation cache as the stationary
operand; adj-products consume per-tile PE transposes. Matmuls run in fp16
(PSUM accumulates fp32); collectives run in fp16.
"""

import numpy as np

N_CORES = 8
N = 8192            # both bipartite sides
D = 128             # feature dim
R = N // N_CORES    # 1024 rows per core
NB_I = R // 128     # 8 local i-blocks
NB_J = N // 128     # 64 j-blocks
JC = 1024           # adj DMA chunk width (fp32 stream)

_compiled = None


def _build():
    import concourse.mybir as mybir
    import concourse.tile as tile
    from concourse import bacc
    from concourse.masks import make_identity

    fp32 = mybir.dt.float32
    fp16 = mybir.dt.float16

    nc = bacc.Bacc("TRN2", target_bir_lowering=False, debug=False,
                   num_devices=N_CORES)

    adj_in = nc.dram_tensor("adj_m", [R, N], fp32, kind="ExternalInput")
    u0_in = nc.dram_tensor("u0_m", [R, D], fp32, kind="ExternalInput")
    s0_in = nc.dram_tensor("s0", [N, D], fp32, kind="ExternalInput")
    ws_in = nc.dram_tensor("ws", [D, D], fp32, kind="ExternalInput")
    wu_in = nc.dram_tensor("wu", [D, D], fp32, kind="ExternalInput")
    wst_in = nc.dram_tensor("wst", [D, D], fp32, kind="ExternalInput")
    wut_in = nc.dram_tensor("wut", [D, D], fp32, kind="ExternalInput")

    sub_out = nc.dram_tensor("sub_out", [R, D], fp32, kind="ExternalOutput")
    uni_out = nc.dram_tensor("uni_out", [R, D], fp32, kind="ExternalOutput")

    group = [list(range(N_CORES))]

    with tile.TileContext(nc) as tc:
        with (
            tc.tile_pool(name="const", bufs=1) as const,
            tc.tile_pool(name="cachep", bufs=1) as cachep,
            tc.tile_pool(name="featp", bufs=1) as featp,
            tc.tile_pool(name="stage", bufs=2) as stagep,
            tc.tile_pool(name="small", bufs=1) as small,
            tc.tile_pool(name="ev", bufs=4) as evp,
            tc.tile_pool(name="tt", bufs=4) as ttp,
            # PSUM budget: 8 banks total = pst1(2) + pstt(2) + acc0..3(4)
            tc.tile_pool(name="ps_t1", bufs=2, space="PSUM") as ps_t1p,
            tc.tile_pool(name="ps_tt", bufs=2, space="PSUM") as ps_ttp,
            tc.tile_pool(name="ps_acc", bufs=1, space="PSUM") as ps_accp,
            tc.tile_pool(name="dram", bufs=1, space="DRAM") as dram,
        ):
            ident16 = const.tile([128, 128], fp16)
            make_identity(nc, ident16)
            ident32 = const.tile([128, 128], fp32)
            make_identity(nc, ident32)

            # ---- weights: Wcs = Ws@Wu@Ws, Wcu = Wu@Ws@Wu (fp32 on PE) ----
            ws_sb = small.tile([D, D], fp32)
            wu_sb = small.tile([D, D], fp32)
            wst_sb = small.tile([D, D], fp32)
            wut_sb = small.tile([D, D], fp32)
            nc.sync.dma_start(out=ws_sb[:], in_=ws_in[:])
            nc.sync.dma_start(out=wu_sb[:], in_=wu_in[:])
            nc.sync.dma_start(out=wst_sb[:], in_=wst_in[:])
            nc.sync.dma_start(out=wut_sb[:], in_=wut_in[:])

            def mm_small(lhsT, rhs, nm):
                ps = ps_ttp.tile([128, 128], fp32, name="ps_" + nm, tag="pstt")
                nc.tensor.matmul(out=ps[:], lhsT=lhsT[:], rhs=rhs[:],
                                 start=True, stop=True)
                sb = small.tile([D, D], fp32, name=nm, tag="wtmp", bufs=4)
                nc.vector.tensor_copy(out=sb[:], in_=ps[:])
                return sb

            wus = mm_small(wut_sb, ws_sb, "wus")        # Wu@Ws
            wcs_sb = mm_small(wst_sb, wus, "wcs")       # Ws@Wu@Ws
            wsu = mm_small(wst_sb, wu_sb, "wsu")        # Ws@Wu
            wcu_sb = mm_small(wut_sb, wsu, "wcu")       # Wu@Ws@Wu

            # ---- u0 prep: [128, 8*132] fp16, col 128 of each block = 1.0 ----
            u0e = const.tile([128, NB_I * 132], fp16)
            nc.gpsimd.memset(u0e[:], 0.0)
            u0_st = stagep.tile([128, NB_I * D], fp32, name="u0_st",
                                tag="adjst")
            nc.sync.dma_start(
                out=u0_st[:].rearrange("p (b n) -> p b n", n=D),
                in_=u0_in[:].rearrange("(b p) n -> p b n", p=128),
            )
            for ib in range(NB_I):
                nc.vector.tensor_copy(
                    out=u0e[:, ib * 132:ib * 132 + D],
                    in_=u0_st[:, ib * D:(ib + 1) * D],
                )
                nc.gpsimd.memset(u0e[:, ib * 132 + D:ib * 132 + D + 1], 1.0)

            # ---- persistent buffers ----
            # natural fp16 cache of adj_m: tile (ib, jb) at
            # cache[:, ib*N + jb*128 : +128]   (partition = i, free = j)
            cache = cachep.tile([128, NB_I * N], fp16)
            # featbuf: per jb: [s0[jb] | t1n[jb]] -> cols jb*256 .. jb*256+256
            featbuf = featp.tile([128, NB_J * 256], fp16)
            # tr: per ib: [t2n[ib] | r1n[ib]] -> cols ib*256 .. +256
            tr16 = small.tile([128, NB_I * 256], fp16)

            # collective DRAM buffers
            rs1_in = dram.tile([N, 132], fp32)
            rs1_out = dram.tile([R, 132], fp32)
            ag1_in = dram.tile([R, D], fp16)
            ag1_out = dram.tile([N, D], fp16, addr_space="Shared")
            rs2_in = dram.tile([N, 256], fp32)
            rs2_out = dram.tile([R, 256], fp32)
            ag2_in = dram.tile([R, D], fp16)
            ag2_out = dram.tile([N, D], fp16, addr_space="Shared")

            # =========== P1: stream adj, build cache, t1p = adj^T @ [u0|1] ===
            for g in range(2):              # 2 column groups of 4096
                for ib in range(NB_I):
                    for h in range(4):      # chunks of 1024 within group
                        jc = g * 4 + h
                        st = stagep.tile([128, JC], fp32, name="adjst",
                                         tag="adjst")
                        nc.sync.dma_start(
                            out=st[:],
                            in_=adj_in[ib * 128:(ib + 1) * 128,
                                       jc * JC:(jc + 1) * JC],
                        )
                        nc.vector.tensor_copy(
                            out=cache[:, ib * N + jc * JC:
                                      ib * N + (jc + 1) * JC],
                            in_=st[:],
                        )
                # matmuls for the 32 j-blocks of this column group
                for jj in range(32):
                    jb = g * 32 + jj
                    ps = ps_t1p.tile([128, 132], fp32, name="pst1", tag="pst1")
                    for ib in range(NB_I):
                        nc.tensor.matmul(
                            out=ps[:],
                            lhsT=cache[:, ib * N + jb * 128:
                                       ib * N + jb * 128 + 128],
                            rhs=u0e[:, ib * 132:(ib + 1) * 132],
                            start=(ib == 0), stop=(ib == NB_I - 1),
                        )
                    ev = evp.tile([128, 132], fp32, name="t1ev", tag="t1ev")
                    if jb % 2:
                        nc.scalar.copy(out=ev[:], in_=ps[:])
                    else:
                        nc.vector.tensor_copy(out=ev[:], in_=ps[:])
                    nc.sync.dma_start(
                        out=rs1_in[jb * 128:(jb + 1) * 128, :], in_=ev[:])

            nc.gpsimd.collective_compute(
                "ReduceScatter", mybir.AluOpType.add, replica_groups=group,
                ins=[rs1_in.opt()], outs=[rs1_out.opt()])

            # ---- overlap window: s0 load/cast + rowsum while RS runs ----
            for c in range(8):
                st = stagep.tile([128, 8 * D], fp32, name="s0st", tag="adjst")
                nc.sync.dma_start(
                    out=st[:].rearrange("p (b n) -> p b n", n=D),
                    in_=s0_in[:].rearrange("(b p) n -> p b n", p=128)
                        [:, c * 8:(c + 1) * 8, :],
                )
                for k in range(8):
                    jb = c * 8 + k
                    nc.vector.tensor_copy(
                        out=featbuf[:, jb * 256:jb * 256 + D],
                        in_=st[:, k * D:(k + 1) * D],
                    )

            # rowsum_m from the fp16 cache (fp32 accumulate on DVE)
            rsum = small.tile([128, NB_I], fp32)
            rtmp = small.tile([128, NB_J], fp32, bufs=2)
            for ib in range(NB_I):
                nc.vector.tensor_reduce(
                    out=rtmp[:], axis=mybir.AxisListType.X,
                    in_=cache[:, ib * N:(ib + 1) * N]
                        .rearrange("p (b n) -> p b n", n=128),
                    op=mybir.AluOpType.add)
                nc.vector.tensor_reduce(
                    out=rsum[:, ib:ib + 1], axis=mybir.AxisListType.X,
                    in_=rtmp[:], op=mybir.AluOpType.add)
            irow = small.tile([128, NB_I], fp32)
            nc.vector.reciprocal(out=irow[:], in_=rsum[:])

            # ---- RS1 results: colsum block, t1n, AllGather ----
            csum = small.tile([128, NB_I], fp32)
            nc.sync.dma_start(
                out=csum[:].rearrange("p (b n) -> p b n", n=1),
                in_=rs1_out[:, D:D + 1].rearrange("(b p) n -> p b n", p=128))
            icol = small.tile([128, NB_I], fp32)
            nc.vector.reciprocal(out=icol[:], in_=csum[:])

            rs1b = stagep.tile([128, NB_I * D], fp32, name="rs1b",
                               tag="adjst")
            nc.sync.dma_start(
                out=rs1b[:].rearrange("p (b n) -> p b n", n=D),
                in_=rs1_out[:, 0:D].rearrange("(b p) n -> p b n", p=128))
            t1nloc = small.tile([128, NB_I * D], fp16)
            for ib in range(NB_I):
                nc.vector.tensor_scalar_mul(
                    out=t1nloc[:, ib * D:(ib + 1) * D],
                    in0=rs1b[:, ib * D:(ib + 1) * D],
                    scalar1=icol[:, ib:ib + 1])
            nc.sync.dma_start(
                out=ag1_in[:].rearrange("(b p) n -> p b n", p=128),
                in_=t1nloc[:].rearrange("p (b n) -> p b n", n=D))
            nc.gpsimd.collective_compute(
                "AllGather", mybir.AluOpType.bypass, replica_groups=group,
                ins=[ag1_in.opt()], outs=[ag1_out.opt()])
            nc.sync.dma_start(
                out=featbuf[:].rearrange("p (b n) -> p b n", n=256)[:, :, D:],
                in_=ag1_out[:].rearrange("(b p) n -> p b n", p=128))

            # =========== P2: r1_m | t2_m = adj_m @ [s0 | t1n] ================
            # accumulators: 4 psum banks [128,512]; ib uses
            # accs[ib//2][:, (ib%2)*256 : +256]; cols 0:128 = r1, 128:256 = t2
            accs = [ps_accp.tile([128, 512], fp32, name=f"acc{i}",
                                 tag=f"acc{i}") for i in range(4)]
            for ib in range(NB_I):
                acc = accs[ib // 2][:, (ib % 2) * 256:(ib % 2) * 256 + 256]
                for jb in range(NB_J):
                    pst = ps_ttp.tile([128, 128], fp16, name="pstt", tag="pstt")
                    nc.tensor.transpose(
                        out=pst[:],
                        in_=cache[:, ib * N + jb * 128:ib * N + jb * 128 + 128],
                        identity=ident16[:])
                    tt = ttp.tile([128, 128], fp16, name="ttsb", tag="ttsb")
                    if jb % 2:
                        nc.scalar.copy(out=tt[:], in_=pst[:])
                    else:
                        nc.vector.tensor_copy(out=tt[:], in_=pst[:])
                    nc.tensor.matmul(
                        out=acc, lhsT=tt[:],
                        rhs=featbuf[:, jb * 256:(jb + 1) * 256],
                        start=(jb == 0), stop=(jb == NB_J - 1))
                # evict + scale into tr: [t2n | r1n]
                nc.vector.tensor_scalar_mul(
                    out=tr16[:, ib * 256:ib * 256 + D],
                    in0=acc[:, D:2 * D], scalar1=irow[:, ib:ib + 1])   # t2n
                nc.vector.tensor_scalar_mul(
                    out=tr16[:, ib * 256 + D:ib * 256 + 2 * D],
                    in0=acc[:, 0:D], scalar1=irow[:, ib:ib + 1])       # r1n

            # =========== P3: t3p | r2p = adj_m^T @ [t2n | r1n] ===============
            for jb in range(NB_J):
                ps = ps_t1p.tile([128, 256], fp32, name="ps3", tag="pst1")
                for ib in range(NB_I):
                    nc.tensor.matmul(
                        out=ps[:],
                        lhsT=cache[:, ib * N + jb * 128:ib * N + jb * 128 + 128],
                        rhs=tr16[:, ib * 256:(ib + 1) * 256],
                        start=(ib == 0), stop=(ib == NB_I - 1))
                ev = evp.tile([128, 256], fp32, name="t3ev", tag="t1ev")
                if jb % 2:
                    nc.scalar.copy(out=ev[:], in_=ps[:])
                else:
                    nc.vector.tensor_copy(out=ev[:], in_=ps[:])
                nc.sync.dma_start(
                    out=rs2_in[jb * 128:(jb + 1) * 128, :], in_=ev[:])

            nc.gpsimd.collective_compute(
                "ReduceScatter", mybir.AluOpType.add, replica_groups=group,
                ins=[rs2_in.opt()], outs=[rs2_out.opt()])

            # t3n (fp32, for final weights), r2n (fp16, for AllGather)
            t3n = small.tile([128, NB_I * D], fp32)
            r2nloc = small.tile([128, NB_I * D], fp16)
            for half in range(2):
                rs2b = stagep.tile([128, 4 * 256], fp32, name="rs2b",
                                   tag="adjst")
                nc.sync.dma_start(
                    out=rs2b[:].rearrange("p (b n) -> p b n", n=256),
                    in_=rs2_out[half * 512:(half + 1) * 512, :]
                        .rearrange("(b p) n -> p b n", p=128))
                for k in range(4):
                    ib = half * 4 + k
                    nc.vector.tensor_scalar_mul(
                        out=t3n[:, ib * D:(ib + 1) * D],
                        in0=rs2b[:, k * 256:k * 256 + D],
                        scalar1=icol[:, ib:ib + 1])
                    nc.vector.tensor_scalar_mul(
                        out=r2nloc[:, ib * D:(ib + 1) * D],
                        in0=rs2b[:, k * 256 + D:k * 256 + 2 * D],
                        scalar1=icol[:, ib:ib + 1])
            nc.sync.dma_start(
                out=ag2_in[:].rearrange("(b p) n -> p b n", p=128),
                in_=r2nloc[:].rearrange("p (b n) -> p b n", n=D))
            nc.gpsimd.collective_compute(
                "AllGather", mybir.AluOpType.bypass, replica_groups=group,
                ins=[ag2_in.opt()], outs=[ag2_out.opt()])
            # r2n full reuses the featbuf slot (dead after P2)
            r2n16 = featp.tile([128, NB_J * D], fp16, name="r2n16",
                               tag="featbuf")
            nc.sync.dma_start(
                out=r2n16[:].rearrange("p (b n) -> p b n", n=D),
                in_=ag2_out[:].rearrange("(b p) n -> p b n", p=128))

            # ---- final sub_out while AG2 runs: sub = t3n @ Wcs (fp32) ------
            for ib in range(NB_I):
                pstf = ps_ttp.tile([128, 128], fp32, name="pstf", tag="pstt")
                nc.tensor.transpose(
                    out=pstf[:], in_=t3n[:, ib * D:(ib + 1) * D],
                    identity=ident32[:])
                t3nT = ttp.tile([128, 128], fp32, name="t3nT", tag="t3nT")
                nc.vector.tensor_copy(out=t3nT[:], in_=pstf[:])
                pso = ps_t1p.tile([128, 128], fp32, name="pso", tag="pst1")
                nc.tensor.matmul(out=pso[:], lhsT=t3nT[:], rhs=wcs_sb[:],
                                 start=True, stop=True)
                osb = evp.tile([128, 128], fp32, name="osb", tag="osb")
                nc.vector.tensor_copy(out=osb[:], in_=pso[:])
                nc.sync.dma_start(
                    out=sub_out[ib * 128:(ib + 1) * 128, :], in_=osb[:])

            # =========== P4: r3_m = adj_m @ r2n ==============================
            acc4s = [ps_accp.tile([128, 512], fp32, name=f"acc4_{i}",
                                  tag=f"acc{i}") for i in range(2)]
            for ib in range(NB_I):
                acc = acc4s[ib // 4][:, (ib % 4) * 128:(ib % 4) * 128 + 128]
                for jb in range(NB_J):
                    pst = ps_ttp.tile([128, 128], fp16, name="pstt4",
                                      tag="pstt")
                    nc.tensor.transpose(
                        out=pst[:],
                        in_=cache[:, ib * N + jb * 128:ib * N + jb * 128 + 128],
                        identity=ident16[:])
                    tt = ttp.tile([128, 128], fp16, name="ttsb4", tag="ttsb")
                    if jb % 2:
                        nc.scalar.copy(out=tt[:], in_=pst[:])
                    else:
                        nc.vector.tensor_copy(out=tt[:], in_=pst[:])
                    nc.tensor.matmul(
                        out=acc, lhsT=tt[:],
                        rhs=r2n16[:, jb * D:(jb + 1) * D],
                        start=(jb == 0), stop=(jb == NB_J - 1))
                # r3n fp32, then uni_out = r3n @ Wcu
                r3n = small.tile([128, 128], fp32, name="r3n", tag="r3n",
                                 bufs=3)
                nc.vector.tensor_scalar_mul(
                    out=r3n[:], in0=acc, scalar1=irow[:, ib:ib + 1])
                pstf = ps_ttp.tile([128, 128], fp32, name="pstf4", tag="pstt")
                nc.tensor.transpose(out=pstf[:], in_=r3n[:],
                                    identity=ident32[:])
                r3nT = ttp.tile([128, 128], fp32, name="r3nT", tag="t3nT")
                nc.vector.tensor_copy(out=r3nT[:], in_=pstf[:])
                pso = ps_t1p.tile([128, 128], fp32, name="pso4", tag="pst1")
                nc.tensor.matmul(out=pso[:], lhsT=r3nT[:], rhs=wcu_sb[:],
                                 start=True, stop=True)
                osb = evp.tile([128, 128], fp32, name="osb4", tag="osb")
                nc.vector.tensor_copy(out=osb[:], in_=pso[:])
                nc.sync.dma_start(
                    out=uni_out[ib * 128:(ib + 1) * 128, :], in_=osb[:])

    nc.compile()
    return nc


def kernel(adj, uni_feat, sub_feat, w_s1, w_u1):
    global _compiled
    from concourse.bass_utils import run_bass_kernel_spmd

    if _compiled is None:
        _compiled = _build()
    nc = _compiled

    adj = np.ascontiguousarray(adj, dtype=np.float32)
    u0 = np.ascontiguousarray(uni_feat, dtype=np.float32)
    s0 = np.ascontiguousarray(sub_feat, dtype=np.float32)
    ws = np.ascontiguousarray(w_s1, dtype=np.float32)
    wu = np.ascontiguousarray(w_u1, dtype=np.float32)
    wst = np.ascontiguousarray(ws.T)
    wut = np.ascontiguousarray(wu.T)

    in_maps = []
    for c in range(N_CORES):
        sl = slice(c * R, (c + 1) * R)
        in_maps.append({
            "adj_m": adj[sl], "u0_m": u0[sl], "s0": s0,
            "ws": ws, "wu": wu, "wst": wst, "wut": wut,
        })

    res = run_bass_kernel_spmd(nc, in_maps, list(range(N_CORES)))
    sub = np.concatenate([res.results[c]["sub_out"] for c in range(N_CORES)],
                         axis=0)
    uni = np.concatenate([res.results[c]["uni_out"] for c in range(N_CORES)],
                         axis=0)
    return sub, uni
